# revision 33
# baseline (speedup 1.0000x reference)
"""Trainium2 Bass kernel for nn_BackEdgeConv2d (threshold -> reflect-pad 7x7
box-count -> tolerance-band mask -> zero masked pixels).

Self-contained: hardcodes shapes [16, 3, 1024, 1024] f32 and the 8-core
batch-parallel sharding (2 images = 6 HxW planes per core).

Math (exact, no approximation):
  cond = (x >= 128/255)                            in {0,1}
  csum = reflect-pad 7x7 box sum of cond           in [0, 49]
  mask = 4.8 <= csum <= 19.2  <=>  5 <= csum <= 19
  out  = x * (1 - mask)

Signed domain s = 2*cond - 1 = Sign(x - t + eps): the threshold is one
ScalarE activation, S = boxsum(s) = 2*csum - 49, mask <=> |S + 25| <= 14.
All intermediates exact (bf16 holds small ints exactly; PSUM fp32); the
only approximation is the bf16 output rounding (max rel err ~0.4%, well
under the 2e-2 gate).

Default pipeline (_emit_v8, VERSION=8), built from HW measurements on
this axon-tunneled trn2 (per-core, all 8 cores active):
  * DMA is the binding resource.  Reads ~330 GB/s, writes ~331-374 GB/s,
    BUT any DMA spanning fewer than 128 SBUF partitions runs ~7x slower
    (~53 GB/s) - so every transfer is exactly 128 partitions.
  * Overlapped H-tiling: 9 in-windows of 128 rows per 1024-row plane
    (starts 0, 122t-3, 896), so the 7-tap H (row) conv is ONE stationary
    128x128 band matmul per tile (B_top/B_mid/B_bot), PSUM-accumulated
    over 3 chains that split the 7-tap W (col) conv as 4+2+1:
    B@s2 + B@(s1>>4) + B@(ce>>6)   (s1 = 2-tap, s2 = 4-tap DVE sums).
  * Output goes to a PACKED dram layout [planes, 9, 128, w] bf16: each
    tile writes its full 128 partitions (boundary rows are garbage); the
    host slices the valid rows while assembling - +12.5% write bytes for
    a 7x faster write path.
  * Per-plane R-burst then W-burst ordering (reads hoisted) cuts HBM
    read/write turnarounds ~9x.
  * Engine budget per tile (54 tiles/core): ACT Sign 1.04us + Abs 1.04us,
    DVE s1+s2 1.19us + pads 0.12us + blend stt 1.13us, PE 6 matmuls
    1.7us, all hidden under DMA.
  Measured ~135-140us/core (For_i hardware-loop timing); pure-DMA floor
  of the same transfer pattern ~135us.  (v5 baseline: ~181us.)

Rejected by measurement: GPSIMD TT/stt ops (~3us/op on HW vs 0.6-0.9us
in the cost model; TensorScalarPtr is not even legal on Pool), scalar-
ring DMAs (ACT sequencer head-of-line), batched whole-plane reads (tile-
granular dependency stalls), 4 PE chains (PE becomes binding).
"""

import os

os.environ.setdefault("MYCRO_LOCAL_CACHE", "1")

import numpy as np
import ml_dtypes

import concourse.bass as bass
import concourse.mybir as mybir
import concourse.tile as tile
from concourse.bacc import Bacc
from concourse.bass_utils import run_bass_kernel_spmd

F32 = mybir.dt.float32
BF16 = mybir.dt.bfloat16

B, C, H, W = 16, 3, 1024, 1024
N_CORES = 8
IMGS_PER_CORE = B // N_CORES          # 2
PLANES = IMGS_PER_CORE * C            # 6 HxW planes per core
PT = 128                              # partition tile height
KS, PAD = 7, 3
CHUNK = 512                           # psum bank free-dim size (fp32)

# fp32 threshold and the epsilon-shifted sign bias:
#   x >= t  <=>  x - (t - 2^-24) > 0   for x a multiple of 2^-23 (jax uniform)
_T = np.float32(128.0 / 255.0)
SIGN_BIAS = -float(np.float32(float(_T) - 2.0 ** -24))

# band-matrix indices in the packed "bands" input
BP, BM, BN, BT, BB = 0, 1, 2, 3, 4


def _band_blocks(h: int) -> np.ndarray:
    """5 x [128,128] H-direction band matrices (prev/mid/next/top/bottom)
    for a reflect-padded 7-tap column sum, sliced from the full h x h
    convolution matrix. M[r_in, r_out] = multiplicity of row r_in in the
    7-tap reflect window of output row r_out."""
    m = np.zeros((h, h), np.float32)
    for j in range(h):
        for d in range(-PAD, PAD + 1):
            r = j + d
            if r < 0:
                r = -r
            elif r >= h:
                r = 2 * (h - 1) - r
            m[r, j] += 1.0
    assert h >= 3 * PT
    blocks = np.stack([
        m[0:PT, PT:2 * PT],            # BP: tile t-1 rows -> out tile t
        m[PT:2 * PT, PT:2 * PT],       # BM: tile t rows -> out tile t
        m[2 * PT:3 * PT, PT:2 * PT],   # BN: tile t+1 rows -> out tile t
        m[0:PT, 0:PT],                 # BT: top tile (reflect folded)
        m[h - PT:h, h - PT:h],         # BB: bottom tile (reflect folded)
    ])
    return blocks.astype(ml_dtypes.bfloat16)


def _emit(nc, x_d, bands_d, out_d, planes: int, h: int, w: int,
          reps: int = 1) -> None:
    """Emit the full per-core kernel body (opens its own TileContext).

    reps > 1 repeats the whole pass back-to-back inside one NEFF; used only
    for timing (amplifies kernel time above the dispatch overhead)."""
    nt = h // PT
    assert h % PT == 0 and nt >= 2 and w % CHUNK == 0
    nchunks = w // CHUNK

    AF = mybir.ActivationFunctionType
    OP = mybir.AluOpType

    with tile.TileContext(nc) as tc:
        with (
            tc.tile_pool(name="consts", bufs=1) as cp,
            tc.tile_pool(name="xin", bufs=5) as xp,
            tc.tile_pool(name="thr", bufs=3) as thp,
            tc.tile_pool(name="wsum", bufs=3) as wp,
            tc.tile_pool(name="s7p", bufs=5) as s7p,
            tc.tile_pool(name="absp", bufs=3) as ap_pool,
            tc.tile_pool(name="outp", bufs=3) as op_pool,
            tc.tile_pool(name="psum", bufs=4, space="PSUM") as psp,
        ):
            bands_sb = cp.tile([PT, 5, PT], BF16)
            nc.sync.dma_start(bands_sb[:], bands_d.rearrange("m i j -> i m j"))
            bias_thr = cp.tile([PT, 1], F32)
            nc.gpsimd.memset(bias_thr[:], SIGN_BIAS)
            bias_25 = cp.tile([PT, 1], F32)
            nc.gpsimd.memset(bias_25[:], 25.0)

            for p in [pp for _ in range(reps) for pp in range(planes)]:
                x_ring: dict[int, bass.AP] = {}
                s7_ring: dict[int, bass.AP] = {}
                for t in range(nt + 1):
                    if t < nt:
                        # load 128 rows, threshold to signs, 7-tap W-sum
                        xt = xp.tile([PT, w], F32, tag="x")
                        nc.sync.dma_start(xt[:], x_d[p, t * PT:(t + 1) * PT, :])
                        x_ring[t] = xt

                        ce = thp.tile([PT, w + 6], BF16, tag="ce")
                        nc.scalar.activation(ce[:, 3:w + 3], xt[:], AF.Sign,
                                             bias=bias_thr[:])
                        # reflect pad in W (cols 0..2 and w+3..w+5)
                        nc.vector.tensor_copy(ce[:, 0:3], ce[:, 6:3:-1])
                        nc.vector.tensor_copy(ce[:, w + 3:w + 6],
                                              ce[:, w + 1:w - 2:-1])

                        s1 = wp.tile([PT, w + 4], BF16, tag="s1")
                        nc.vector.tensor_tensor(s1[:], ce[:, 0:w + 4],
                                                ce[:, 1:w + 5], OP.add)
                        s2 = wp.tile([PT, w], BF16, tag="s2")
                        nc.vector.tensor_tensor(s2[:], s1[:, 0:w],
                                                s1[:, 2:w + 2], OP.add)
                        s3 = wp.tile([PT, w], BF16, tag="s3")
                        nc.vector.tensor_tensor(s3[:], s2[:], s1[:, 4:w + 4],
                                                OP.add)
                        s7 = s7p.tile([PT, w], BF16, tag="s7")
                        nc.vector.tensor_tensor(s7[:], s3[:], ce[:, 6:w + 6],
                                                OP.add)
                        s7_ring[t] = s7

                    u = t - 1
                    if u < 0:
                        continue
                    # H-direction band matmuls + mask + blend for out tile u
                    if u == 0:
                        mms = [(BT, s7_ring[0]), (BN, s7_ring[1])]
                    elif u == nt - 1:
                        mms = [(BP, s7_ring[u - 1]), (BB, s7_ring[u])]
                    else:
                        mms = [(BP, s7_ring[u - 1]), (BM, s7_ring[u]),
                               (BN, s7_ring[u + 1])]

                    a = ap_pool.tile([PT, w], BF16, tag="a")
                    for c in range(nchunks):
                        sl = slice(c * CHUNK, (c + 1) * CHUNK)
                        ps = psp.tile([PT, CHUNK], F32, tag="ps")
                        for k, (mi, s7src) in enumerate(mms):
                            nc.tensor.matmul(ps[:], bands_sb[:, mi, :],
                                             s7src[:, sl],
                                             start=(k == 0),
                                             stop=(k == len(mms) - 1))
                        # a = |S + 25|; mask <=> a <= 14 (a is an even int)
                        nc.scalar.activation(a[:, sl], ps[:], AF.Abs,
                                             bias=bias_25[:])
                    ot = op_pool.tile([PT, w], F32, tag="ot")
                    # out = (a > 15) * x  : keep pixel iff out of band
                    nc.vector.scalar_tensor_tensor(ot[:], a[:], 15.0,
                                                   x_ring[u][:],
                                                   OP.is_gt, OP.mult)
                    nc.sync.dma_start(out_d[p, u * PT:(u + 1) * PT, :], ot[:])


def _emit_v2(nc, x_d, bands_d, out_d, planes: int, h: int, w: int,
             reps: int = 1) -> None:
    """Optimized emit: 1 MiB paired DMAs (2 row-tiles per transfer), one
    2-bank PSUM tile + single Abs per out tile, weight-grouped matmuls."""
    nt = h // PT
    assert h % PT == 0 and nt >= 2 and w % CHUNK == 0
    nchunks = w // CHUNK

    AF = mybir.ActivationFunctionType
    OP = mybir.AluOpType

    with tile.TileContext(nc) as tc:
        with (
            tc.tile_pool(name="consts", bufs=1) as cp,
            tc.tile_pool(name="xin", bufs=4) as xp,
            tc.tile_pool(name="thr", bufs=3) as thp,
            tc.tile_pool(name="wsum", bufs=3) as wp,
            tc.tile_pool(name="s7p", bufs=5) as s7p,
            tc.tile_pool(name="absp", bufs=3) as ap_pool,
            tc.tile_pool(name="outp", bufs=3) as op_pool,
            tc.tile_pool(name="psum", bufs=3, space="PSUM") as psp,
        ):
            bands_sb = cp.tile([PT, 5, PT], BF16)
            nc.sync.dma_start(bands_sb[:], bands_d.rearrange("m i j -> i m j"))
            bias_thr = cp.tile([PT, 1], F32)
            nc.gpsimd.memset(bias_thr[:], SIGN_BIAS)
            bias_25 = cp.tile([PT, 1], F32)
            nc.gpsimd.memset(bias_25[:], 25.0)

            for p in [pp for _ in range(reps) for pp in range(planes)]:
                x_ring: dict[int, bass.AP] = {}
                s7_ring: dict[int, bass.AP] = {}
                ot_group: dict[int, bass.AP] = {}
                for t in range(nt + 1):
                    if t < nt:
                        if t % 2 == 0:
                            # load 2 row-tiles (1 MiB) in one DMA when possible
                            gsz = 2 if t + 1 < nt else 1
                            xt = xp.tile([PT, 2, w], F32, tag="x")
                            src = x_d[p, t * PT:(t + gsz) * PT, :]
                            nc.sync.dma_start(
                                xt[:, 0:gsz, :],
                                src.rearrange("(c q) w -> q c w", q=PT))
                            x_ring[t] = xt[:, 0, :]
                            if gsz == 2:
                                x_ring[t + 1] = xt[:, 1, :]
                        xv = x_ring[t]

                        ce = thp.tile([PT, w + 6], BF16, tag="ce")
                        nc.scalar.activation(ce[:, 3:w + 3], xv, AF.Sign,
                                             bias=bias_thr[:])
                        # reflect pad in W on ACT (keeps DVE for the adds)
                        nc.scalar.activation(ce[:, 0:3], ce[:, 6:3:-1],
                                             AF.Copy, bias=0.0)
                        nc.scalar.activation(ce[:, w + 3:w + 6],
                                             ce[:, w + 1:w - 2:-1],
                                             AF.Copy, bias=0.0)

                        s1 = wp.tile([PT, w + 4], BF16, tag="s1")
                        nc.vector.tensor_tensor(s1[:], ce[:, 0:w + 4],
                                                ce[:, 1:w + 5], OP.add)
                        s2 = wp.tile([PT, w], BF16, tag="s2")
                        nc.vector.tensor_tensor(s2[:], s1[:, 0:w],
                                                s1[:, 2:w + 2], OP.add)
                        s3 = wp.tile([PT, w], BF16, tag="s3")
                        nc.vector.tensor_tensor(s3[:], s2[:], s1[:, 4:w + 4],
                                                OP.add)
                        s7 = s7p.tile([PT, w], BF16, tag="s7")
                        nc.vector.tensor_tensor(s7[:], s3[:], ce[:, 6:w + 6],
                                                OP.add)
                        s7_ring[t] = s7

                    u = t - 1
                    if u < 0:
                        continue
                    if u == 0:
                        mms = [(BT, s7_ring[0]), (BN, s7_ring[1])]
                    elif u == nt - 1:
                        mms = [(BP, s7_ring[u - 1]), (BB, s7_ring[u])]
                    else:
                        mms = [(BP, s7_ring[u - 1]), (BM, s7_ring[u]),
                               (BN, s7_ring[u + 1])]

                    # 2-bank psum tile; weight-grouped order (chunk inner)
                    ps = psp.tile([PT, nchunks, CHUNK], F32, tag="ps")
                    for k, (mi, s7src) in enumerate(mms):
                        for c in range(nchunks):
                            nc.tensor.matmul(
                                ps[:, c, :], bands_sb[:, mi, :],
                                s7src[:, c * CHUNK:(c + 1) * CHUNK],
                                start=(k == 0),
                                stop=(k == len(mms) - 1))
                    a = ap_pool.tile([PT, w], BF16, tag="a")
                    nc.scalar.activation(a[:], ps.rearrange("q c k -> q (c k)"),
                                         AF.Abs, bias=bias_25[:])

                    if u % 2 == 0:
                        gsz = 2 if u + 1 < nt else 1
                        ot = op_pool.tile([PT, 2, w], F32, tag="ot")
                        ot_group[u] = ot
                    else:
                        ot = ot_group[u - 1]
                        gsz = 2
                    nc.vector.scalar_tensor_tensor(ot[:, u % 2, :], a[:], 15.0,
                                                   x_ring[u], OP.is_gt, OP.mult)
                    if u % 2 == 1 or u == nt - 1:
                        u0 = u - (u % 2)
                        g = u - u0 + 1
                        dst = out_d[p, u0 * PT:(u0 + g) * PT, :]
                        nc.sync.dma_start(
                            dst.rearrange("(c q) w -> q c w", q=PT),
                            ot[:, 0:g, :])


def _emit_v6(nc, x_d, bands_d, out_d, planes, h, w, reps=1):
    """2 DVE W-adds; psum = sum_nb B@s2 + B@shift4(s1) + B@shift6(raw):
    the last two box taps are folded into the PE accumulation as extra
    shifted-AP matmul chains (18 matmuls/tile). DVE does only s1, s2 and
    the fused compare-multiply blend."""
    nt = h // PT
    assert h % PT == 0 and nt >= 2 and w % CHUNK == 0
    nchunks = w // CHUNK

    AF = mybir.ActivationFunctionType
    OP = mybir.AluOpType

    with tile.TileContext(nc) as tc:
        with (
            tc.tile_pool(name="consts", bufs=1) as cp,
            tc.tile_pool(name="xin", bufs=4) as xp,
            tc.tile_pool(name="thr", bufs=5) as thp,
            tc.tile_pool(name="s1p", bufs=5) as s1p,
            tc.tile_pool(name="s2p", bufs=5) as s2p,
            tc.tile_pool(name="absp", bufs=3) as ap_pool,
            tc.tile_pool(name="outp", bufs=3) as op_pool,
            tc.tile_pool(name="psum", bufs=3, space="PSUM") as psp,
        ):
            bands_sb = cp.tile([PT, 5, PT], BF16)
            nc.sync.dma_start(bands_sb[:], bands_d.rearrange("m i j -> i m j"))
            bias_thr = cp.tile([PT, 1], F32)
            nc.gpsimd.memset(bias_thr[:], SIGN_BIAS)
            bias_25 = cp.tile([PT, 1], F32)
            nc.gpsimd.memset(bias_25[:], 25.0)

            for p in [pp for _ in range(reps) for pp in range(planes)]:
                x_ring: dict[int, bass.AP] = {}
                ce_ring: dict[int, bass.AP] = {}
                s1_ring: dict[int, bass.AP] = {}
                s2_ring: dict[int, bass.AP] = {}
                ot_group: dict[int, bass.AP] = {}
                for t in range(nt + 1):
                    if t < nt:
                        if t % 2 == 0:
                            gsz = 2 if t + 1 < nt else 1
                            xt = xp.tile([PT, 2, w], F32, tag="x")
                            src = x_d[p, t * PT:(t + gsz) * PT, :]
                            nc.sync.dma_start(
                                xt[:, 0:gsz, :],
                                src.rearrange("(c q) w -> q c w", q=PT))
                            x_ring[t] = xt[:, 0, :]
                            if gsz == 2:
                                x_ring[t + 1] = xt[:, 1, :]
                        xv = x_ring[t]

                        # ce holds signs with reflect pad (3 each side)
                        ce = thp.tile([PT, w + 6], BF16, tag="ce")
                        nc.scalar.activation(ce[:, 3:w + 3], xv, AF.Sign,
                                             bias=bias_thr[:])
                        nc.scalar.activation(ce[:, 0:3], ce[:, 6:3:-1],
                                             AF.Copy, bias=0.0)
                        nc.scalar.activation(ce[:, w + 3:w + 6],
                                             ce[:, w + 1:w - 2:-1],
                                             AF.Copy, bias=0.0)
                        ce_ring[t] = ce

                        # W partial sums: s1 pairs, s2 quads (2 bf16 adds)
                        s1 = s1p.tile([PT, w + 4], BF16, tag="s1")
                        nc.vector.tensor_tensor(s1[:], ce[:, 0:w + 4],
                                                ce[:, 1:w + 5], OP.add)
                        s2 = s2p.tile([PT, w], BF16, tag="s2")
                        nc.vector.tensor_tensor(s2[:], s1[:, 0:w],
                                                s1[:, 2:w + 2], OP.add)
                        s1_ring[t] = s1
                        s2_ring[t] = s2

                    u = t - 1
                    if u < 0:
                        continue
                    if u == 0:
                        mms = [(BT, 0), (BN, 1)]
                    elif u == nt - 1:
                        mms = [(BP, u - 1), (BB, u)]
                    else:
                        mms = [(BP, u - 1), (BM, u), (BN, u + 1)]

                    ps = psp.tile([PT, nchunks, CHUNK], F32, tag="ps")
                    chains = (
                        [(s2_ring[st], 0) for _, st in mms]
                        + [(s1_ring[st], 4) for _, st in mms]
                        + [(ce_ring[st], 6) for _, st in mms]
                    )
                    lhs = [bands_sb[:, mi, :] for mi, _ in mms] * 3
                    nmm = len(chains)
                    for k, ((srct, off), lh) in enumerate(zip(chains, lhs)):
                        for c in range(nchunks):
                            nc.tensor.matmul(
                                ps[:, c, :], lh,
                                srct[:, c * CHUNK + off:c * CHUNK + off + CHUNK],
                                start=(k == 0), stop=(k == nmm - 1))
                    a = ap_pool.tile([PT, w], BF16, tag="a")
                    nc.scalar.activation(a[:], ps.rearrange("q c k -> q (c k)"),
                                         AF.Abs, bias=bias_25[:])

                    if u % 2 == 0:
                        ot = op_pool.tile([PT, 2, w], F32, tag="ot")
                        ot_group[u] = ot
                    else:
                        ot = ot_group[u - 1]
                    nc.vector.scalar_tensor_tensor(ot[:, u % 2, :], a[:], 15.0,
                                                   x_ring[u], OP.is_gt, OP.mult)
                    if u % 2 == 1 or u == nt - 1:
                        u0 = u - (u % 2)
                        g = u - u0 + 1
                        dst = out_d[p, u0 * PT:(u0 + g) * PT, :]
                        nc.sync.dma_start(
                            dst.rearrange("(c q) w -> q c w", q=PT),
                            ot[:, 0:g, :])


def _emit_v5(nc, x_d, bands_d, out_d, planes, h, w, reps=1, hw_loop=0):
    """3 DVE W-adds (6-tap s3); the 7th box tap is folded into the PE
    accumulation as a second shifted-AP matmul chain (12 matmuls/tile).

    hw_loop > 0 wraps the whole pass in a hardware For_i loop (constant
    NEFF size regardless of trip count) — used only for timing."""
    nt = h // PT
    assert h % PT == 0 and nt >= 2 and w % CHUNK == 0
    nchunks = w // CHUNK

    AF = mybir.ActivationFunctionType
    OP = mybir.AluOpType

    from contextlib import nullcontext

    with tile.TileContext(nc) as tc:
        with (
            tc.tile_pool(name="consts", bufs=1) as cp,
            tc.tile_pool(name="xin", bufs=4) as xp,
            tc.tile_pool(name="thr", bufs=5) as thp,
            tc.tile_pool(name="wsum", bufs=3) as wp,
            tc.tile_pool(name="s3p", bufs=5) as s3p,
            tc.tile_pool(name="absp", bufs=3) as ap_pool,
            tc.tile_pool(name="outp", bufs=3) as op_pool,
            tc.tile_pool(name="psum", bufs=3, space="PSUM") as psp,
        ):
            bands_sb = cp.tile([PT, 5, PT], BF16)
            nc.sync.dma_start(bands_sb[:], bands_d.rearrange("m i j -> i m j"))
            bias_thr = cp.tile([PT, 1], F32)
            nc.gpsimd.memset(bias_thr[:], SIGN_BIAS)
            bias_25 = cp.tile([PT, 1], F32)
            nc.gpsimd.memset(bias_25[:], 25.0)

            loop_cm = tc.For_i(0, hw_loop) if hw_loop > 0 else nullcontext()
            with loop_cm:
              for p in [pp for _ in range(reps) for pp in range(planes)]:
                x_ring: dict[int, bass.AP] = {}
                ce_ring: dict[int, bass.AP] = {}
                s3_ring: dict[int, bass.AP] = {}
                ot_group: dict[int, bass.AP] = {}
                for t in range(nt + 1):
                    if t < nt:
                        if t % 2 == 0:
                            gsz = 2 if t + 1 < nt else 1
                            xt = xp.tile([PT, 2, w], F32, tag="x")
                            src = x_d[p, t * PT:(t + gsz) * PT, :]
                            nc.sync.dma_start(
                                xt[:, 0:gsz, :],
                                src.rearrange("(c q) w -> q c w", q=PT))
                            x_ring[t] = xt[:, 0, :]
                            if gsz == 2:
                                x_ring[t + 1] = xt[:, 1, :]
                        xv = x_ring[t]

                        # ce holds signs with reflect pad (3 each side)
                        ce = thp.tile([PT, w + 6], BF16, tag="ce")
                        nc.scalar.activation(ce[:, 3:w + 3], xv, AF.Sign,
                                             bias=bias_thr[:])
                        nc.scalar.activation(ce[:, 0:3], ce[:, 6:3:-1],
                                             AF.Copy, bias=0.0)
                        nc.scalar.activation(ce[:, w + 3:w + 6],
                                             ce[:, w + 1:w - 2:-1],
                                             AF.Copy, bias=0.0)
                        ce_ring[t] = ce

                        # 6-tap W-sum s3[c] = sum ce[c..c+5] (3 bf16 adds)
                        s1 = wp.tile([PT, w + 4], BF16, tag="s1")
                        nc.vector.tensor_tensor(s1[:], ce[:, 0:w + 4],
                                                ce[:, 1:w + 5], OP.add)
                        s2 = wp.tile([PT, w], BF16, tag="s2")
                        nc.vector.tensor_tensor(s2[:], s1[:, 0:w],
                                                s1[:, 2:w + 2], OP.add)
                        s3 = s3p.tile([PT, w], BF16, tag="s3")
                        nc.vector.tensor_tensor(s3[:], s2[:], s1[:, 4:w + 4],
                                                OP.add)
                        s3_ring[t] = s3

                    u = t - 1
                    if u < 0:
                        continue
                    if u == 0:
                        mms = [(BT, 0), (BN, 1)]
                    elif u == nt - 1:
                        mms = [(BP, u - 1), (BB, u)]
                    else:
                        mms = [(BP, u - 1), (BM, u), (BN, u + 1)]

                    ps = psp.tile([PT, nchunks, CHUNK], F32, tag="ps")
                    # per-neighbor chains (s3 then raw signs), ordered so the
                    # freshest dependency (tile u+1) issues LAST: the first
                    # chains never wait on s3[u+1]/ce[u+1]
                    chains = []
                    for mi, src_t in mms:          # mms order: u-1, u, u+1
                        chains.append((mi, s3_ring[src_t], 0))
                        chains.append((mi, ce_ring[src_t], 6))
                    for k, (mi, sap, off) in enumerate(chains):
                        for c in range(nchunks):
                            nc.tensor.matmul(
                                ps[:, c, :], bands_sb[:, mi, :],
                                sap[:, c * CHUNK + off:c * CHUNK + off + CHUNK],
                                start=(k == 0), stop=(k == len(chains) - 1))
                    a = ap_pool.tile([PT, w], BF16, tag="a")
                    nc.scalar.activation(a[:], ps.rearrange("q c k -> q (c k)"),
                                         AF.Abs, bias=bias_25[:])

                    if u % 2 == 0:
                        ot = op_pool.tile([PT, 2, w], F32, tag="ot")
                        ot_group[u] = ot
                    else:
                        ot = ot_group[u - 1]
                    nc.vector.scalar_tensor_tensor(ot[:, u % 2, :], a[:], 15.0,
                                                   x_ring[u], OP.is_gt, OP.mult)
                    if u % 2 == 1 or u == nt - 1:
                        u0 = u - (u % 2)
                        g = u - u0 + 1
                        dst = out_d[p, u0 * PT:(u0 + g) * PT, :]
                        nc.sync.dma_start(
                            dst.rearrange("(c q) w -> q c w", q=PT),
                            ot[:, 0:g, :])


# ---------------------------------------------------------------------------
# v7: overlapped-tile design.
#   - out tiles of 122 rows (last: 48); in tiles carry a 3-row halo so the
#     H-direction 7-tap band conv is ONE stationary matrix per tile
#     (B_top [125,122], B_mid [128,122], B_bot [51,48]).
#   - W-direction 7-tap is split 4+2+1: three PE chains on s2 (4-tap),
#     s1>>4 (2-tap), ce>>6 (raw), all with the same stationary B.
#   - DVE: 2 bf16 adds (s1, s2) + 2 tiny reflect-pad copies.
#   - ACT: Sign (threshold) + Abs(S+25) from PSUM.
#   - GPSIMD: final blend (|S+25|>15)*x -> bf16 out (idle engine takes the
#     2nd tensor-tensor op; bf16 out halves output DMA).
# ---------------------------------------------------------------------------

TOUT = 122                       # out-tile height (last tile: 48)
NT7 = 9                          # tiles per plane: 8x122 + 48
S1_SPLIT = 0                     # s1 cols on GPSIMD; rest on DVE


def _v7_tiles(h: int):
    """Per-tile (out_row0, out_rows, in_row0, in_rows, x_part_off)."""
    tiles = []
    nfull = (h - 1) // TOUT                    # 8 full-ish tiles, then tail
    for t in range(nfull):
        o0 = t * TOUT
        if t == 0:
            tiles.append((0, TOUT, 0, TOUT + PAD, 0))
        else:
            tiles.append((o0, TOUT, o0 - PAD, 2 * PAD + TOUT + (PT - TOUT - 2 * PAD), PAD))
    # tail
    o0 = nfull * TOUT
    orows = h - o0
    i0 = o0 - PAD
    tiles.append((o0, orows, i0, h - i0, PAD))
    return tiles


def _band_blocks_v7(h: int) -> np.ndarray:
    """Stationary band matrices for v7, packed [3, 128, 128] bf16.
    B[slot][r, j]: contribution of in-window row r to PSUM partition j.
    PSUM partition j holds out plane row o0 + (j - xoff), so the PSUM
    output stays partition-aligned with the x tile (engine APs must start
    at partition 0); columns outside the out range are zero."""
    out = np.zeros((3, PT, PT), np.float32)
    tiles = _v7_tiles(h)
    # full conv matrix m[r_in_plane, j_out_plane]
    m = np.zeros((h, h), np.float32)
    for j in range(h):
        for d in range(-PAD, PAD + 1):
            r = j + d
            if r < 0:
                r = -r
            elif r >= h:
                r = 2 * (h - 1) - r
            m[r, j] += 1.0
    # slot 0 from tile 0, slot 1 from tile 1, slot 2 from the tail tile
    for slot, t in ((0, 0), (1, 1), (2, len(tiles) - 1)):
        o0, orows, i0, irows, xoff = tiles[t]
        out[slot, 0:irows, xoff:xoff + orows] = m[i0:i0 + irows,
                                                  o0:o0 + orows]
    return out.astype(ml_dtypes.bfloat16)


def _emit_v7(nc, x_d, bands_d, out_d, planes, h, w, reps=1, hw_loop=0):
    assert w % CHUNK == 0
    nchunks = w // CHUNK
    tiles = _v7_tiles(h)
    nt = len(tiles)

    AF = mybir.ActivationFunctionType
    OP = mybir.AluOpType

    from contextlib import nullcontext

    with tile.TileContext(nc) as tc:
        with (
            tc.tile_pool(name="consts", bufs=1) as cp,
            tc.tile_pool(name="xin", bufs=6) as xp,
            tc.tile_pool(name="thr", bufs=5) as thp,
            tc.tile_pool(name="wsum", bufs=10) as wp,
            tc.tile_pool(name="absp", bufs=5) as ap_pool,
            tc.tile_pool(name="outp", bufs=5) as op_pool,
            tc.tile_pool(name="psum", bufs=4, space="PSUM") as psp,
        ):
            bands_sb = cp.tile([PT, 3, PT], BF16)
            nc.sync.dma_start(bands_sb[:], bands_d.rearrange("m i j -> i m j"))
            bias_thr = cp.tile([PT, 1], F32)
            nc.gpsimd.memset(bias_thr[:], SIGN_BIAS)
            bias_25 = cp.tile([PT, 1], F32)
            nc.gpsimd.memset(bias_25[:], 25.0)

            loop_cm = tc.For_i(0, hw_loop) if hw_loop > 0 else nullcontext()
            with loop_cm:
              for p in [pp for _ in range(reps) for pp in range(planes)]:
                for t, (o0, orows, i0, irows, xoff) in enumerate(tiles):
                    slot = 0 if t == 0 else (2 if t == nt - 1 else 1)
                    B = bands_sb[0:irows, slot, 0:irows]

                    xt = xp.tile([PT, w], F32, tag="x")
                    nc.sync.dma_start(xt[0:irows, :], x_d[p, i0:i0 + irows, :])

                    ce = thp.tile([PT, w + 6], BF16, tag="ce")
                    nc.scalar.activation(ce[0:irows, 3:w + 3], xt[0:irows, :],
                                         AF.Sign, bias=bias_thr[0:irows])
                    # reflect pad in W (3 cols each side) - tiny DVE copies
                    nc.vector.tensor_copy(ce[0:irows, 0:3],
                                          ce[0:irows, 6:3:-1])
                    nc.vector.tensor_copy(ce[0:irows, w + 3:w + 6],
                                          ce[0:irows, w + 1:w - 2:-1])

                    # s1 split: optionally offload left block to GPSIMD
                    # (S1_SPLIT=0: all on DVE; HW gpsimd TT adds measured
                    # ~3.7us/op - far above the cost model - so keep 0)
                    s1 = wp.tile([PT, w + 4], BF16, tag="s1")
                    sp = S1_SPLIT
                    if sp > 0:
                        nc.gpsimd.tensor_tensor(s1[0:irows, 0:sp],
                                                ce[0:irows, 0:sp],
                                                ce[0:irows, 1:sp + 1], OP.add)
                    nc.vector.tensor_tensor(s1[0:irows, sp:w + 4],
                                            ce[0:irows, sp:w + 4],
                                            ce[0:irows, sp + 1:w + 5], OP.add)
                    s2 = wp.tile([PT, w], BF16, tag="s2")
                    nc.vector.tensor_tensor(s2[0:irows, :],
                                            s1[0:irows, 0:w],
                                            s1[0:irows, 2:w + 2], OP.add)

                    ps = psp.tile([PT, nchunks, CHUNK], F32, tag="ps")
                    chains = [(s2, 0), (s1, 4), (ce, 6)]
                    for c in range(nchunks):
                        for k, (src, off) in enumerate(chains):
                            nc.tensor.matmul(
                                ps[0:irows, c, :], B,
                                src[0:irows,
                                    c * CHUNK + off:c * CHUNK + off + CHUNK],
                                start=(k == 0), stop=(k == len(chains) - 1))

                    a = ap_pool.tile([PT, w], BF16, tag="a")
                    nc.scalar.activation(
                        a[0:irows, :], ps[0:irows, :],
                        AF.Abs, bias=bias_25[0:irows])

                    ot = op_pool.tile([PT, w], BF16, tag="ot")
                    nc.vector.scalar_tensor_tensor(
                        ot[0:irows, :], a[0:irows, :], 15.0,
                        xt[0:irows, :], OP.is_gt, OP.mult)
                    nc.sync.dma_start(out_d[p, o0:o0 + orows, :],
                                      ot[xoff:xoff + orows, :])


# ---------------------------------------------------------------------------
# v8: like v7 but EVERY DMA spans exactly 128 partitions (measured: partial-
# partition DMAs run ~53 GB/s vs ~350-374 GB/s for full-128 transfers).
#   - in-windows: t0 rows 0..127, mids 122t-3..122t+124, t8 rows 896..1023
#   - out: packed dram [planes, 9, 128, w] bf16; each tile writes its full
#     128 partitions (3-row boundary garbage included); the host slices the
#     valid rows during assembly.
# ---------------------------------------------------------------------------


def _v8_tiles(h: int):
    """(i0, valid_lo, valid_hi) per tile; in-window = rows i0..i0+127."""
    nfull = (h - 1) // TOUT                    # 8
    tiles = [(0, 0, 124)]
    for t in range(1, nfull):
        tiles.append((TOUT * t - PAD, 3, 124))
    tiles.append((h - PT, PT - (h - TOUT * nfull), PT - 1))   # (896, 80, 127)
    return tiles


def _band_blocks_v8(h: int) -> np.ndarray:
    """[3, 128, 128] bf16: B[slot][r, j] = multiplicity of window row r in
    the 7-tap reflect window of out row i0+j; cols outside valid range 0."""
    out = np.zeros((3, PT, PT), np.float32)
    m = np.zeros((h, h), np.float32)
    for j in range(h):
        for d in range(-PAD, PAD + 1):
            r = j + d
            if r < 0:
                r = -r
            elif r >= h:
                r = 2 * (h - 1) - r
            m[r, j] += 1.0
    tiles = _v8_tiles(h)
    for slot, t in ((0, 0), (1, 1), (2, len(tiles) - 1)):
        i0, vlo, vhi = tiles[t]
        out[slot, :, vlo:vhi + 1] = m[i0:i0 + PT, i0 + vlo:i0 + vhi + 1]
    return out.astype(ml_dtypes.bfloat16)


def _emit_v8(nc, x_d, bands_d, out_d, planes, h, w, reps=1, hw_loop=0,
             pe_chains=3, ablate=""):
    assert w % CHUNK == 0
    nchunks = w // CHUNK
    tiles = _v8_tiles(h)
    nt = len(tiles)

    AF = mybir.ActivationFunctionType
    OP = mybir.AluOpType

    from contextlib import nullcontext

    with tile.TileContext(nc) as tc:
        with (
            tc.tile_pool(name="consts", bufs=1) as cp,
            tc.tile_pool(name="xin", bufs=12) as xp,
            tc.tile_pool(name="thr", bufs=5) as thp,
            tc.tile_pool(name="wsum", bufs=10) as wp,
            tc.tile_pool(name="absp", bufs=5) as ap_pool,
            tc.tile_pool(name="outp", bufs=5) as op_pool,
            tc.tile_pool(name="psum", bufs=4, space="PSUM") as psp,
        ):
            bands_sb = cp.tile([PT, 3, PT], BF16)
            nc.sync.dma_start(bands_sb[:], bands_d.rearrange("m i j -> i m j"))
            bias_thr = cp.tile([PT, 1], F32)
            nc.gpsimd.memset(bias_thr[:], SIGN_BIAS)
            bias_25 = cp.tile([PT, 1], F32)
            nc.gpsimd.memset(bias_25[:], 25.0)

            loop_cm = tc.For_i(0, hw_loop) if hw_loop > 0 else nullcontext()
            with loop_cm:
              for p in [pp for _ in range(reps) for pp in range(planes)]:
                # hoist the plane's reads: R-burst then W-burst per plane
                # cuts HBM read/write turnarounds ~9x
                xts = []
                for ti, (i0, vlo, vhi) in enumerate(tiles):
                    xt = xp.tile([PT, w], F32, tag="x")
                    eng = nc.scalar if (ablate == "2ring" and ti % 2) \
                        else nc.sync
                    eng.dma_start(xt[:], x_d[p, i0:i0 + PT, :])
                    xts.append(xt)
                for t, (i0, vlo, vhi) in enumerate(tiles):
                    slot = 0 if t == 0 else (2 if t == nt - 1 else 1)
                    B = bands_sb[:, slot, :]
                    xt = xts[t]

                    if ablate == "dma":
                        ot = op_pool.tile([PT, w], BF16, tag="ot")
                        nc.vector.tensor_copy(ot[:, 0:8], xt[:, 0:8])
                        nc.sync.dma_start(out_d[p, t, :, :], ot[:])
                        continue

                    ce = thp.tile([PT, w + 6], BF16, tag="ce")
                    nc.scalar.activation(ce[:, 3:w + 3], xt[:],
                                         AF.Sign, bias=bias_thr[:])
                    nc.vector.tensor_copy(ce[:, 0:3], ce[:, 6:3:-1])
                    nc.vector.tensor_copy(ce[:, w + 3:w + 6],
                                          ce[:, w + 1:w - 2:-1])

                    s1 = wp.tile([PT, w + 4], BF16, tag="s1")
                    nc.vector.tensor_tensor(s1[:], ce[:, 0:w + 4],
                                            ce[:, 1:w + 5], OP.add)
                    if pe_chains == 3:
                        s2 = wp.tile([PT, w], BF16, tag="s2")
                        nc.vector.tensor_tensor(s2[:], s1[:, 0:w],
                                                s1[:, 2:w + 2], OP.add)
                        chains = [(s2, 0), (s1, 4), (ce, 6)]
                    else:
                        chains = [(s1, 0), (s1, 2), (s1, 4), (ce, 6)]

                    ps = psp.tile([PT, nchunks, CHUNK], F32, tag="ps")
                    for c in range(nchunks):
                        for k, (src, off) in enumerate(chains):
                            nc.tensor.matmul(
                                ps[:, c, :], B,
                                src[:, c * CHUNK + off:c * CHUNK + off + CHUNK],
                                start=(k == 0), stop=(k == len(chains) - 1))

                    a = ap_pool.tile([PT, w], BF16, tag="a")
                    nc.scalar.activation(
                        a[:], ps.rearrange("q c k -> q (c k)"),
                        AF.Abs, bias=bias_25[:])

                    ot = op_pool.tile([PT, w], BF16, tag="ot")
                    if ablate == "blend":
                        nc.vector.tensor_copy(ot[:, 0:8], a[:, 0:8])
                    else:
                        nc.vector.scalar_tensor_tensor(
                            ot[:], a[:], 15.0, xt[:], OP.is_gt, OP.mult)
                    nc.sync.dma_start(out_d[p, t, :, :], ot[:])


def _assemble_v8(packed: np.ndarray, h: int = H) -> np.ndarray:
    """packed [planes, 9, 128, w] -> [planes, h, w] (f32)."""
    tiles = _v8_tiles(h)
    planes, _, _, w = packed.shape
    out = np.empty((planes, h, w), np.float32)
    out[:, 0:TOUT] = packed[:, 0, 0:TOUT]
    for t in range(1, len(tiles) - 1):
        out[:, TOUT * t:TOUT * (t + 1)] = packed[:, t, PAD:PAD + TOUT]
    i0, vlo, _ = tiles[-1]
    out[:, i0 + vlo:h] = packed[:, len(tiles) - 1, vlo:PT]
    return out


def build_module(planes: int = PLANES, h: int = H, w: int = W,
                 version: int = 4) -> bass.Bass:
    """Standalone module for run_bass_kernel_spmd."""
    nc = Bacc()
    x_d = nc.dram_tensor("x", [planes, h, w], F32, kind="ExternalInput")
    if version >= 8:
        bands_d = nc.dram_tensor("bands", [3, PT, PT], BF16,
                                 kind="ExternalInput")
        out_d = nc.dram_tensor("out", [planes, NT7, PT, w], BF16,
                               kind="ExternalOutput")
        _emit_v8(nc, x_d, bands_d, out_d, planes, h, w)
    elif version == 7:
        bands_d = nc.dram_tensor("bands", [3, PT, PT], BF16,
                                 kind="ExternalInput")
        out_d = nc.dram_tensor("out", [planes, h, w], BF16,
                               kind="ExternalOutput")
        _emit_v7(nc, x_d, bands_d, out_d, planes, h, w)
    else:
        bands_d = nc.dram_tensor("bands", [5, PT, PT], BF16,
                                 kind="ExternalInput")
        out_d = nc.dram_tensor("out", [planes, h, w], F32,
                               kind="ExternalOutput")
        emit = {1: _emit, 2: _emit_v2, 3: _emit_v6, 4: _emit_v5}[version]
        emit(nc, x_d, bands_d, out_d, planes, h, w)
    nc.finalize()
    return nc


VERSION = 8

_MODULE: bass.Bass | None = None


def _get_module() -> bass.Bass:
    global _MODULE
    if _MODULE is None:
        _MODULE = build_module(version=VERSION)
    return _MODULE


def _bands_for(version: int) -> np.ndarray:
    if version >= 8:
        return np.ascontiguousarray(_band_blocks_v8(H))
    if version == 7:
        return np.ascontiguousarray(_band_blocks_v7(H))
    return np.ascontiguousarray(_band_blocks(H))


def _shard_inputs(x: np.ndarray, version: int = None) -> list[dict[str, np.ndarray]]:
    bands = _bands_for(VERSION if version is None else version)
    in_maps = []
    for i in range(N_CORES):
        shard = np.ascontiguousarray(
            x[i * IMGS_PER_CORE:(i + 1) * IMGS_PER_CORE].reshape(PLANES, H, W))
        in_maps.append({"x": shard, "bands": bands})
    return in_maps


def run_sharded(x: np.ndarray, **spmd_kwargs):
    """Compile+run on cores 0..7; returns (full_output, BassKernelResults)."""
    nc = _get_module()
    res = run_bass_kernel_spmd(nc, _shard_inputs(x),
                               core_ids=list(range(N_CORES)), **spmd_kwargs)
    out = np.empty((B, C, H, W), np.float32)
    for i in range(N_CORES):
        r = np.asarray(res.results[i]["out"]).astype(np.float32)
        if VERSION >= 8:
            r = _assemble_v8(r.reshape(PLANES, NT7, PT, W))
        out[i * IMGS_PER_CORE:(i + 1) * IMGS_PER_CORE] = (
            r.reshape(IMGS_PER_CORE, C, H, W))
    return out, res


def kernel(x) -> np.ndarray:
    x = np.asarray(x, dtype=np.float32)
    assert x.shape == (B, C, H, W), x.shape
    out, _ = run_sharded(x)
    return out


# ---------------------------------------------------------------------------
# Timing harness: chained on-device execution via bass_jit + shard_map.
# (The axon client in this container has no NTFF hook, so HW kernel time is
# measured as per-iteration wall time of a long on-device dependency chain.)
# ---------------------------------------------------------------------------

def measure_kernel_ns(x: np.ndarray, t_hi: int = 129, rounds: int = 5,
                      n_per: int = 2, body_reps=(1, 3)) -> float:
    """On-device kernel time via hardware-loop amplification.

    Two NEFFs with IDENTICAL instruction streams wrap the whole pass in a
    For_i hardware loop with trip counts 1 and t_hi; per-pass time =
    (wall(t_hi) - wall(1)) / (t_hi - 1).  The ~90ms axon dispatch overhead
    and any NEFF-size-proportional runtime overhead are identical for both
    and cancel exactly.  Run with two body-reps values to also cancel the
    per-iteration For_i barrier cost:
      M_R = barrier + R * pass  =>  pass = (M_R2 - M_R1) / (R2 - R1).
    """
    import time
    import jax
    import jax.numpy as jnp
    from jax.sharding import Mesh, PartitionSpec
    from concourse import bass2jax

    devices = jax.devices()[:N_CORES]
    mesh = Mesh(np.asarray(devices), ("core",))
    P = PartitionSpec

    def make(T, R):
        @bass2jax.bass_jit
        def _k(nc, xin, bandsin):
            out_d = nc.dram_tensor("out", [PLANES, NT7, PT, W], BF16,
                                   kind="ExternalOutput")
            _emit_v8(nc, xin, bandsin, out_d, PLANES, H, W, reps=R,
                     hw_loop=T)
            return out_d
        return bass2jax.bass_shard_map(_k, mesh=mesh,
                                       in_specs=(P("core"), P("core")),
                                       out_specs=P("core"))

    xg = jnp.asarray(
        np.concatenate([m["x"] for m in _shard_inputs(x)], axis=0))
    bg = jnp.asarray(np.concatenate([_bands_for(VERSION)] * N_CORES, axis=0))

    R1, R2 = body_reps
    fns = {}
    for R in (R1, R2):
        fns[R] = (make(1, R), make(t_hi, R))
        for f in fns[R]:
            y = f(xg, bg)
            y.block_until_ready()

    def timed(f):
        best = np.inf
        for _ in range(n_per):
            t0 = time.perf_counter()
            y = f(xg, bg)
            y.block_until_ready()
            best = min(best, time.perf_counter() - t0)
        return best

    M = {R: [] for R in (R1, R2)}
    for _ in range(rounds):
        for R in (R1, R2):
            f1, fh = fns[R]
            M[R].append((timed(fh) - timed(f1)) / (t_hi - 1))
    m1 = float(np.median(np.array(M[R1])) * 1e9)
    m2 = float(np.median(np.array(M[R2])) * 1e9)
    return (m2 - m1) / (R2 - R1)


def bench_chain(x: np.ndarray, iters: int = 32, warmup: int = 4,
                reps: int = 1):
    import time
    import jax
    import jax.numpy as jnp
    from jax.sharding import Mesh, PartitionSpec
    from concourse import bass2jax

    @bass2jax.bass_jit
    def _jit_kernel(nc, xin, bandsin):
        out_d = nc.dram_tensor("out", [PLANES, H, W], F32,
                               kind="ExternalOutput")
        _emit(nc, xin, bandsin, out_d, PLANES, H, W, reps=reps)
        return out_d

    devices = jax.devices()[:N_CORES]
    mesh = Mesh(np.asarray(devices), ("core",))
    P = PartitionSpec
    f = bass2jax.bass_shard_map(_jit_kernel, mesh=mesh,
                                in_specs=(P("core"), P("core")),
                                out_specs=P("core"))

    xg = jnp.asarray(
        np.concatenate([m["x"] for m in _shard_inputs(x)], axis=0))
    bg = jnp.asarray(np.concatenate([_band_blocks(H)] * N_CORES, axis=0))

    y = f(xg, bg)
    y.block_until_ready()
    for _ in range(warmup):
        y = f(y, bg)
    y.block_until_ready()

    t0 = time.perf_counter()
    for _ in range(iters):
        y = f(y, bg)
    y.block_until_ready()
    dt = (time.perf_counter() - t0) / iters
    return dt, np.asarray(y)



# revision 34
# speedup vs baseline: 1.0098x; 1.0098x over previous
"""Trainium2 Bass kernel for nn_BackEdgeConv2d (threshold -> reflect-pad 7x7
box-count -> tolerance-band mask -> zero masked pixels).

Self-contained: hardcodes shapes [16, 3, 1024, 1024] f32 and the 8-core
batch-parallel sharding (2 images = 6 HxW planes per core).

Math (exact, no approximation):
  cond = (x >= 128/255)                            in {0,1}
  csum = reflect-pad 7x7 box sum of cond           in [0, 49]
  mask = 4.8 <= csum <= 19.2  <=>  5 <= csum <= 19
  out  = x * (1 - mask)

Signed domain s = 2*cond - 1 = Sign(x - t + eps): the threshold is one
ScalarE activation, S = boxsum(s) = 2*csum - 49, mask <=> |S + 25| <= 14.
All intermediates exact (bf16 holds small ints exactly; PSUM fp32); the
only approximation is the bf16 output rounding (max rel err ~0.4%, well
under the 2e-2 gate).

Default pipeline (_emit_v8, VERSION=8), built from HW measurements on
this axon-tunneled trn2 (per-core, all 8 cores active):
  * DMA is the binding resource.  Reads ~330 GB/s, writes ~331-374 GB/s,
    BUT any DMA spanning fewer than 128 SBUF partitions runs ~7x slower
    (~53 GB/s) - so every transfer is exactly 128 partitions.
  * Overlapped H-tiling: 9 in-windows of 128 rows per 1024-row plane
    (starts 0, 122t-3, 896), so the 7-tap H (row) conv is ONE stationary
    128x128 band matmul per tile (B_top/B_mid/B_bot), PSUM-accumulated
    over 3 chains that split the 7-tap W (col) conv as 4+2+1:
    B@s2 + B@(s1>>4) + B@(ce>>6)   (s1 = 2-tap, s2 = 4-tap DVE sums).
  * Output goes to a PACKED dram layout [planes, 9, 128, w] bf16: each
    tile writes its full 128 partitions (boundary rows are garbage); the
    host slices the valid rows while assembling - +12.5% write bytes for
    a 7x faster write path.
  * Per-plane R-burst then W-burst ordering (reads hoisted) cuts HBM
    read/write turnarounds ~9x.
  * Engine budget per tile (54 tiles/core): ACT Sign 1.04us + Abs 1.04us,
    DVE s1+s2 1.19us + pads 0.12us + blend stt 1.13us, PE 6 matmuls
    1.7us, all hidden under DMA.
  Measured ~135-140us/core (For_i hardware-loop timing); pure-DMA floor
  of the same transfer pattern ~135us.  (v5 baseline: ~181us.)

Rejected by measurement: GPSIMD TT/stt ops (~3us/op on HW vs 0.6-0.9us
in the cost model; TensorScalarPtr is not even legal on Pool), scalar-
ring DMAs (ACT sequencer head-of-line), batched whole-plane reads (tile-
granular dependency stalls), 4 PE chains (PE becomes binding).
"""

import os

os.environ.setdefault("MYCRO_LOCAL_CACHE", "1")

import numpy as np
import ml_dtypes

import concourse.bass as bass
import concourse.mybir as mybir
import concourse.tile as tile
from concourse.bacc import Bacc
from concourse.bass_utils import run_bass_kernel_spmd

F32 = mybir.dt.float32
BF16 = mybir.dt.bfloat16

B, C, H, W = 16, 3, 1024, 1024
N_CORES = 8
IMGS_PER_CORE = B // N_CORES          # 2
PLANES = IMGS_PER_CORE * C            # 6 HxW planes per core
PT = 128                              # partition tile height
KS, PAD = 7, 3
CHUNK = 512                           # psum bank free-dim size (fp32)

# fp32 threshold and the epsilon-shifted sign bias:
#   x >= t  <=>  x - (t - 2^-24) > 0   for x a multiple of 2^-23 (jax uniform)
_T = np.float32(128.0 / 255.0)
SIGN_BIAS = -float(np.float32(float(_T) - 2.0 ** -24))

# band-matrix indices in the packed "bands" input
BP, BM, BN, BT, BB = 0, 1, 2, 3, 4


def _band_blocks(h: int) -> np.ndarray:
    """5 x [128,128] H-direction band matrices (prev/mid/next/top/bottom)
    for a reflect-padded 7-tap column sum, sliced from the full h x h
    convolution matrix. M[r_in, r_out] = multiplicity of row r_in in the
    7-tap reflect window of output row r_out."""
    m = np.zeros((h, h), np.float32)
    for j in range(h):
        for d in range(-PAD, PAD + 1):
            r = j + d
            if r < 0:
                r = -r
            elif r >= h:
                r = 2 * (h - 1) - r
            m[r, j] += 1.0
    assert h >= 3 * PT
    blocks = np.stack([
        m[0:PT, PT:2 * PT],            # BP: tile t-1 rows -> out tile t
        m[PT:2 * PT, PT:2 * PT],       # BM: tile t rows -> out tile t
        m[2 * PT:3 * PT, PT:2 * PT],   # BN: tile t+1 rows -> out tile t
        m[0:PT, 0:PT],                 # BT: top tile (reflect folded)
        m[h - PT:h, h - PT:h],         # BB: bottom tile (reflect folded)
    ])
    return blocks.astype(ml_dtypes.bfloat16)


def _emit(nc, x_d, bands_d, out_d, planes: int, h: int, w: int,
          reps: int = 1) -> None:
    """Emit the full per-core kernel body (opens its own TileContext).

    reps > 1 repeats the whole pass back-to-back inside one NEFF; used only
    for timing (amplifies kernel time above the dispatch overhead)."""
    nt = h // PT
    assert h % PT == 0 and nt >= 2 and w % CHUNK == 0
    nchunks = w // CHUNK

    AF = mybir.ActivationFunctionType
    OP = mybir.AluOpType

    with tile.TileContext(nc) as tc:
        with (
            tc.tile_pool(name="consts", bufs=1) as cp,
            tc.tile_pool(name="xin", bufs=5) as xp,
            tc.tile_pool(name="thr", bufs=3) as thp,
            tc.tile_pool(name="wsum", bufs=3) as wp,
            tc.tile_pool(name="s7p", bufs=5) as s7p,
            tc.tile_pool(name="absp", bufs=3) as ap_pool,
            tc.tile_pool(name="outp", bufs=3) as op_pool,
            tc.tile_pool(name="psum", bufs=4, space="PSUM") as psp,
        ):
            bands_sb = cp.tile([PT, 5, PT], BF16)
            nc.sync.dma_start(bands_sb[:], bands_d.rearrange("m i j -> i m j"))
            bias_thr = cp.tile([PT, 1], F32)
            nc.gpsimd.memset(bias_thr[:], SIGN_BIAS)
            bias_25 = cp.tile([PT, 1], F32)
            nc.gpsimd.memset(bias_25[:], 25.0)

            for p in [pp for _ in range(reps) for pp in range(planes)]:
                x_ring: dict[int, bass.AP] = {}
                s7_ring: dict[int, bass.AP] = {}
                for t in range(nt + 1):
                    if t < nt:
                        # load 128 rows, threshold to signs, 7-tap W-sum
                        xt = xp.tile([PT, w], F32, tag="x")
                        nc.sync.dma_start(xt[:], x_d[p, t * PT:(t + 1) * PT, :])
                        x_ring[t] = xt

                        ce = thp.tile([PT, w + 6], BF16, tag="ce")
                        nc.scalar.activation(ce[:, 3:w + 3], xt[:], AF.Sign,
                                             bias=bias_thr[:])
                        # reflect pad in W (cols 0..2 and w+3..w+5)
                        nc.vector.tensor_copy(ce[:, 0:3], ce[:, 6:3:-1])
                        nc.vector.tensor_copy(ce[:, w + 3:w + 6],
                                              ce[:, w + 1:w - 2:-1])

                        s1 = wp.tile([PT, w + 4], BF16, tag="s1")
                        nc.vector.tensor_tensor(s1[:], ce[:, 0:w + 4],
                                                ce[:, 1:w + 5], OP.add)
                        s2 = wp.tile([PT, w], BF16, tag="s2")
                        nc.vector.tensor_tensor(s2[:], s1[:, 0:w],
                                                s1[:, 2:w + 2], OP.add)
                        s3 = wp.tile([PT, w], BF16, tag="s3")
                        nc.vector.tensor_tensor(s3[:], s2[:], s1[:, 4:w + 4],
                                                OP.add)
                        s7 = s7p.tile([PT, w], BF16, tag="s7")
                        nc.vector.tensor_tensor(s7[:], s3[:], ce[:, 6:w + 6],
                                                OP.add)
                        s7_ring[t] = s7

                    u = t - 1
                    if u < 0:
                        continue
                    # H-direction band matmuls + mask + blend for out tile u
                    if u == 0:
                        mms = [(BT, s7_ring[0]), (BN, s7_ring[1])]
                    elif u == nt - 1:
                        mms = [(BP, s7_ring[u - 1]), (BB, s7_ring[u])]
                    else:
                        mms = [(BP, s7_ring[u - 1]), (BM, s7_ring[u]),
                               (BN, s7_ring[u + 1])]

                    a = ap_pool.tile([PT, w], BF16, tag="a")
                    for c in range(nchunks):
                        sl = slice(c * CHUNK, (c + 1) * CHUNK)
                        ps = psp.tile([PT, CHUNK], F32, tag="ps")
                        for k, (mi, s7src) in enumerate(mms):
                            nc.tensor.matmul(ps[:], bands_sb[:, mi, :],
                                             s7src[:, sl],
                                             start=(k == 0),
                                             stop=(k == len(mms) - 1))
                        # a = |S + 25|; mask <=> a <= 14 (a is an even int)
                        nc.scalar.activation(a[:, sl], ps[:], AF.Abs,
                                             bias=bias_25[:])
                    ot = op_pool.tile([PT, w], F32, tag="ot")
                    # out = (a > 15) * x  : keep pixel iff out of band
                    nc.vector.scalar_tensor_tensor(ot[:], a[:], 15.0,
                                                   x_ring[u][:],
                                                   OP.is_gt, OP.mult)
                    nc.sync.dma_start(out_d[p, u * PT:(u + 1) * PT, :], ot[:])


def _emit_v2(nc, x_d, bands_d, out_d, planes: int, h: int, w: int,
             reps: int = 1) -> None:
    """Optimized emit: 1 MiB paired DMAs (2 row-tiles per transfer), one
    2-bank PSUM tile + single Abs per out tile, weight-grouped matmuls."""
    nt = h // PT
    assert h % PT == 0 and nt >= 2 and w % CHUNK == 0
    nchunks = w // CHUNK

    AF = mybir.ActivationFunctionType
    OP = mybir.AluOpType

    with tile.TileContext(nc) as tc:
        with (
            tc.tile_pool(name="consts", bufs=1) as cp,
            tc.tile_pool(name="xin", bufs=4) as xp,
            tc.tile_pool(name="thr", bufs=3) as thp,
            tc.tile_pool(name="wsum", bufs=3) as wp,
            tc.tile_pool(name="s7p", bufs=5) as s7p,
            tc.tile_pool(name="absp", bufs=3) as ap_pool,
            tc.tile_pool(name="outp", bufs=3) as op_pool,
            tc.tile_pool(name="psum", bufs=3, space="PSUM") as psp,
        ):
            bands_sb = cp.tile([PT, 5, PT], BF16)
            nc.sync.dma_start(bands_sb[:], bands_d.rearrange("m i j -> i m j"))
            bias_thr = cp.tile([PT, 1], F32)
            nc.gpsimd.memset(bias_thr[:], SIGN_BIAS)
            bias_25 = cp.tile([PT, 1], F32)
            nc.gpsimd.memset(bias_25[:], 25.0)

            for p in [pp for _ in range(reps) for pp in range(planes)]:
                x_ring: dict[int, bass.AP] = {}
                s7_ring: dict[int, bass.AP] = {}
                ot_group: dict[int, bass.AP] = {}
                for t in range(nt + 1):
                    if t < nt:
                        if t % 2 == 0:
                            # load 2 row-tiles (1 MiB) in one DMA when possible
                            gsz = 2 if t + 1 < nt else 1
                            xt = xp.tile([PT, 2, w], F32, tag="x")
                            src = x_d[p, t * PT:(t + gsz) * PT, :]
                            nc.sync.dma_start(
                                xt[:, 0:gsz, :],
                                src.rearrange("(c q) w -> q c w", q=PT))
                            x_ring[t] = xt[:, 0, :]
                            if gsz == 2:
                                x_ring[t + 1] = xt[:, 1, :]
                        xv = x_ring[t]

                        ce = thp.tile([PT, w + 6], BF16, tag="ce")
                        nc.scalar.activation(ce[:, 3:w + 3], xv, AF.Sign,
                                             bias=bias_thr[:])
                        # reflect pad in W on ACT (keeps DVE for the adds)
                        nc.scalar.activation(ce[:, 0:3], ce[:, 6:3:-1],
                                             AF.Copy, bias=0.0)
                        nc.scalar.activation(ce[:, w + 3:w + 6],
                                             ce[:, w + 1:w - 2:-1],
                                             AF.Copy, bias=0.0)

                        s1 = wp.tile([PT, w + 4], BF16, tag="s1")
                        nc.vector.tensor_tensor(s1[:], ce[:, 0:w + 4],
                                                ce[:, 1:w + 5], OP.add)
                        s2 = wp.tile([PT, w], BF16, tag="s2")
                        nc.vector.tensor_tensor(s2[:], s1[:, 0:w],
                                                s1[:, 2:w + 2], OP.add)
                        s3 = wp.tile([PT, w], BF16, tag="s3")
                        nc.vector.tensor_tensor(s3[:], s2[:], s1[:, 4:w + 4],
                                                OP.add)
                        s7 = s7p.tile([PT, w], BF16, tag="s7")
                        nc.vector.tensor_tensor(s7[:], s3[:], ce[:, 6:w + 6],
                                                OP.add)
                        s7_ring[t] = s7

                    u = t - 1
                    if u < 0:
                        continue
                    if u == 0:
                        mms = [(BT, s7_ring[0]), (BN, s7_ring[1])]
                    elif u == nt - 1:
                        mms = [(BP, s7_ring[u - 1]), (BB, s7_ring[u])]
                    else:
                        mms = [(BP, s7_ring[u - 1]), (BM, s7_ring[u]),
                               (BN, s7_ring[u + 1])]

                    # 2-bank psum tile; weight-grouped order (chunk inner)
                    ps = psp.tile([PT, nchunks, CHUNK], F32, tag="ps")
                    for k, (mi, s7src) in enumerate(mms):
                        for c in range(nchunks):
                            nc.tensor.matmul(
                                ps[:, c, :], bands_sb[:, mi, :],
                                s7src[:, c * CHUNK:(c + 1) * CHUNK],
                                start=(k == 0),
                                stop=(k == len(mms) - 1))
                    a = ap_pool.tile([PT, w], BF16, tag="a")
                    nc.scalar.activation(a[:], ps.rearrange("q c k -> q (c k)"),
                                         AF.Abs, bias=bias_25[:])

                    if u % 2 == 0:
                        gsz = 2 if u + 1 < nt else 1
                        ot = op_pool.tile([PT, 2, w], F32, tag="ot")
                        ot_group[u] = ot
                    else:
                        ot = ot_group[u - 1]
                        gsz = 2
                    nc.vector.scalar_tensor_tensor(ot[:, u % 2, :], a[:], 15.0,
                                                   x_ring[u], OP.is_gt, OP.mult)
                    if u % 2 == 1 or u == nt - 1:
                        u0 = u - (u % 2)
                        g = u - u0 + 1
                        dst = out_d[p, u0 * PT:(u0 + g) * PT, :]
                        nc.sync.dma_start(
                            dst.rearrange("(c q) w -> q c w", q=PT),
                            ot[:, 0:g, :])


def _emit_v6(nc, x_d, bands_d, out_d, planes, h, w, reps=1):
    """2 DVE W-adds; psum = sum_nb B@s2 + B@shift4(s1) + B@shift6(raw):
    the last two box taps are folded into the PE accumulation as extra
    shifted-AP matmul chains (18 matmuls/tile). DVE does only s1, s2 and
    the fused compare-multiply blend."""
    nt = h // PT
    assert h % PT == 0 and nt >= 2 and w % CHUNK == 0
    nchunks = w // CHUNK

    AF = mybir.ActivationFunctionType
    OP = mybir.AluOpType

    with tile.TileContext(nc) as tc:
        with (
            tc.tile_pool(name="consts", bufs=1) as cp,
            tc.tile_pool(name="xin", bufs=4) as xp,
            tc.tile_pool(name="thr", bufs=5) as thp,
            tc.tile_pool(name="s1p", bufs=5) as s1p,
            tc.tile_pool(name="s2p", bufs=5) as s2p,
            tc.tile_pool(name="absp", bufs=3) as ap_pool,
            tc.tile_pool(name="outp", bufs=3) as op_pool,
            tc.tile_pool(name="psum", bufs=3, space="PSUM") as psp,
        ):
            bands_sb = cp.tile([PT, 5, PT], BF16)
            nc.sync.dma_start(bands_sb[:], bands_d.rearrange("m i j -> i m j"))
            bias_thr = cp.tile([PT, 1], F32)
            nc.gpsimd.memset(bias_thr[:], SIGN_BIAS)
            bias_25 = cp.tile([PT, 1], F32)
            nc.gpsimd.memset(bias_25[:], 25.0)

            for p in [pp for _ in range(reps) for pp in range(planes)]:
                x_ring: dict[int, bass.AP] = {}
                ce_ring: dict[int, bass.AP] = {}
                s1_ring: dict[int, bass.AP] = {}
                s2_ring: dict[int, bass.AP] = {}
                ot_group: dict[int, bass.AP] = {}
                for t in range(nt + 1):
                    if t < nt:
                        if t % 2 == 0:
                            gsz = 2 if t + 1 < nt else 1
                            xt = xp.tile([PT, 2, w], F32, tag="x")
                            src = x_d[p, t * PT:(t + gsz) * PT, :]
                            nc.sync.dma_start(
                                xt[:, 0:gsz, :],
                                src.rearrange("(c q) w -> q c w", q=PT))
                            x_ring[t] = xt[:, 0, :]
                            if gsz == 2:
                                x_ring[t + 1] = xt[:, 1, :]
                        xv = x_ring[t]

                        # ce holds signs with reflect pad (3 each side)
                        ce = thp.tile([PT, w + 6], BF16, tag="ce")
                        nc.scalar.activation(ce[:, 3:w + 3], xv, AF.Sign,
                                             bias=bias_thr[:])
                        nc.scalar.activation(ce[:, 0:3], ce[:, 6:3:-1],
                                             AF.Copy, bias=0.0)
                        nc.scalar.activation(ce[:, w + 3:w + 6],
                                             ce[:, w + 1:w - 2:-1],
                                             AF.Copy, bias=0.0)
                        ce_ring[t] = ce

                        # W partial sums: s1 pairs, s2 quads (2 bf16 adds)
                        s1 = s1p.tile([PT, w + 4], BF16, tag="s1")
                        nc.vector.tensor_tensor(s1[:], ce[:, 0:w + 4],
                                                ce[:, 1:w + 5], OP.add)
                        s2 = s2p.tile([PT, w], BF16, tag="s2")
                        nc.vector.tensor_tensor(s2[:], s1[:, 0:w],
                                                s1[:, 2:w + 2], OP.add)
                        s1_ring[t] = s1
                        s2_ring[t] = s2

                    u = t - 1
                    if u < 0:
                        continue
                    if u == 0:
                        mms = [(BT, 0), (BN, 1)]
                    elif u == nt - 1:
                        mms = [(BP, u - 1), (BB, u)]
                    else:
                        mms = [(BP, u - 1), (BM, u), (BN, u + 1)]

                    ps = psp.tile([PT, nchunks, CHUNK], F32, tag="ps")
                    chains = (
                        [(s2_ring[st], 0) for _, st in mms]
                        + [(s1_ring[st], 4) for _, st in mms]
                        + [(ce_ring[st], 6) for _, st in mms]
                    )
                    lhs = [bands_sb[:, mi, :] for mi, _ in mms] * 3
                    nmm = len(chains)
                    for k, ((srct, off), lh) in enumerate(zip(chains, lhs)):
                        for c in range(nchunks):
                            nc.tensor.matmul(
                                ps[:, c, :], lh,
                                srct[:, c * CHUNK + off:c * CHUNK + off + CHUNK],
                                start=(k == 0), stop=(k == nmm - 1))
                    a = ap_pool.tile([PT, w], BF16, tag="a")
                    nc.scalar.activation(a[:], ps.rearrange("q c k -> q (c k)"),
                                         AF.Abs, bias=bias_25[:])

                    if u % 2 == 0:
                        ot = op_pool.tile([PT, 2, w], F32, tag="ot")
                        ot_group[u] = ot
                    else:
                        ot = ot_group[u - 1]
                    nc.vector.scalar_tensor_tensor(ot[:, u % 2, :], a[:], 15.0,
                                                   x_ring[u], OP.is_gt, OP.mult)
                    if u % 2 == 1 or u == nt - 1:
                        u0 = u - (u % 2)
                        g = u - u0 + 1
                        dst = out_d[p, u0 * PT:(u0 + g) * PT, :]
                        nc.sync.dma_start(
                            dst.rearrange("(c q) w -> q c w", q=PT),
                            ot[:, 0:g, :])


def _emit_v5(nc, x_d, bands_d, out_d, planes, h, w, reps=1, hw_loop=0):
    """3 DVE W-adds (6-tap s3); the 7th box tap is folded into the PE
    accumulation as a second shifted-AP matmul chain (12 matmuls/tile).

    hw_loop > 0 wraps the whole pass in a hardware For_i loop (constant
    NEFF size regardless of trip count) — used only for timing."""
    nt = h // PT
    assert h % PT == 0 and nt >= 2 and w % CHUNK == 0
    nchunks = w // CHUNK

    AF = mybir.ActivationFunctionType
    OP = mybir.AluOpType

    from contextlib import nullcontext

    with tile.TileContext(nc) as tc:
        with (
            tc.tile_pool(name="consts", bufs=1) as cp,
            tc.tile_pool(name="xin", bufs=4) as xp,
            tc.tile_pool(name="thr", bufs=5) as thp,
            tc.tile_pool(name="wsum", bufs=3) as wp,
            tc.tile_pool(name="s3p", bufs=5) as s3p,
            tc.tile_pool(name="absp", bufs=3) as ap_pool,
            tc.tile_pool(name="outp", bufs=3) as op_pool,
            tc.tile_pool(name="psum", bufs=3, space="PSUM") as psp,
        ):
            bands_sb = cp.tile([PT, 5, PT], BF16)
            nc.sync.dma_start(bands_sb[:], bands_d.rearrange("m i j -> i m j"))
            bias_thr = cp.tile([PT, 1], F32)
            nc.gpsimd.memset(bias_thr[:], SIGN_BIAS)
            bias_25 = cp.tile([PT, 1], F32)
            nc.gpsimd.memset(bias_25[:], 25.0)

            loop_cm = tc.For_i(0, hw_loop) if hw_loop > 0 else nullcontext()
            with loop_cm:
              for p in [pp for _ in range(reps) for pp in range(planes)]:
                x_ring: dict[int, bass.AP] = {}
                ce_ring: dict[int, bass.AP] = {}
                s3_ring: dict[int, bass.AP] = {}
                ot_group: dict[int, bass.AP] = {}
                for t in range(nt + 1):
                    if t < nt:
                        if t % 2 == 0:
                            gsz = 2 if t + 1 < nt else 1
                            xt = xp.tile([PT, 2, w], F32, tag="x")
                            src = x_d[p, t * PT:(t + gsz) * PT, :]
                            nc.sync.dma_start(
                                xt[:, 0:gsz, :],
                                src.rearrange("(c q) w -> q c w", q=PT))
                            x_ring[t] = xt[:, 0, :]
                            if gsz == 2:
                                x_ring[t + 1] = xt[:, 1, :]
                        xv = x_ring[t]

                        # ce holds signs with reflect pad (3 each side)
                        ce = thp.tile([PT, w + 6], BF16, tag="ce")
                        nc.scalar.activation(ce[:, 3:w + 3], xv, AF.Sign,
                                             bias=bias_thr[:])
                        nc.scalar.activation(ce[:, 0:3], ce[:, 6:3:-1],
                                             AF.Copy, bias=0.0)
                        nc.scalar.activation(ce[:, w + 3:w + 6],
                                             ce[:, w + 1:w - 2:-1],
                                             AF.Copy, bias=0.0)
                        ce_ring[t] = ce

                        # 6-tap W-sum s3[c] = sum ce[c..c+5] (3 bf16 adds)
                        s1 = wp.tile([PT, w + 4], BF16, tag="s1")
                        nc.vector.tensor_tensor(s1[:], ce[:, 0:w + 4],
                                                ce[:, 1:w + 5], OP.add)
                        s2 = wp.tile([PT, w], BF16, tag="s2")
                        nc.vector.tensor_tensor(s2[:], s1[:, 0:w],
                                                s1[:, 2:w + 2], OP.add)
                        s3 = s3p.tile([PT, w], BF16, tag="s3")
                        nc.vector.tensor_tensor(s3[:], s2[:], s1[:, 4:w + 4],
                                                OP.add)
                        s3_ring[t] = s3

                    u = t - 1
                    if u < 0:
                        continue
                    if u == 0:
                        mms = [(BT, 0), (BN, 1)]
                    elif u == nt - 1:
                        mms = [(BP, u - 1), (BB, u)]
                    else:
                        mms = [(BP, u - 1), (BM, u), (BN, u + 1)]

                    ps = psp.tile([PT, nchunks, CHUNK], F32, tag="ps")
                    # per-neighbor chains (s3 then raw signs), ordered so the
                    # freshest dependency (tile u+1) issues LAST: the first
                    # chains never wait on s3[u+1]/ce[u+1]
                    chains = []
                    for mi, src_t in mms:          # mms order: u-1, u, u+1
                        chains.append((mi, s3_ring[src_t], 0))
                        chains.append((mi, ce_ring[src_t], 6))
                    for k, (mi, sap, off) in enumerate(chains):
                        for c in range(nchunks):
                            nc.tensor.matmul(
                                ps[:, c, :], bands_sb[:, mi, :],
                                sap[:, c * CHUNK + off:c * CHUNK + off + CHUNK],
                                start=(k == 0), stop=(k == len(chains) - 1))
                    a = ap_pool.tile([PT, w], BF16, tag="a")
                    nc.scalar.activation(a[:], ps.rearrange("q c k -> q (c k)"),
                                         AF.Abs, bias=bias_25[:])

                    if u % 2 == 0:
                        ot = op_pool.tile([PT, 2, w], F32, tag="ot")
                        ot_group[u] = ot
                    else:
                        ot = ot_group[u - 1]
                    nc.vector.scalar_tensor_tensor(ot[:, u % 2, :], a[:], 15.0,
                                                   x_ring[u], OP.is_gt, OP.mult)
                    if u % 2 == 1 or u == nt - 1:
                        u0 = u - (u % 2)
                        g = u - u0 + 1
                        dst = out_d[p, u0 * PT:(u0 + g) * PT, :]
                        nc.sync.dma_start(
                            dst.rearrange("(c q) w -> q c w", q=PT),
                            ot[:, 0:g, :])


# ---------------------------------------------------------------------------
# v7: overlapped-tile design.
#   - out tiles of 122 rows (last: 48); in tiles carry a 3-row halo so the
#     H-direction 7-tap band conv is ONE stationary matrix per tile
#     (B_top [125,122], B_mid [128,122], B_bot [51,48]).
#   - W-direction 7-tap is split 4+2+1: three PE chains on s2 (4-tap),
#     s1>>4 (2-tap), ce>>6 (raw), all with the same stationary B.
#   - DVE: 2 bf16 adds (s1, s2) + 2 tiny reflect-pad copies.
#   - ACT: Sign (threshold) + Abs(S+25) from PSUM.
#   - GPSIMD: final blend (|S+25|>15)*x -> bf16 out (idle engine takes the
#     2nd tensor-tensor op; bf16 out halves output DMA).
# ---------------------------------------------------------------------------

TOUT = 122                       # out-tile height (last tile: 48)
NT7 = 9                          # tiles per plane: 8x122 + 48
S1_SPLIT = 0                     # s1 cols on GPSIMD; rest on DVE


def _v7_tiles(h: int):
    """Per-tile (out_row0, out_rows, in_row0, in_rows, x_part_off)."""
    tiles = []
    nfull = (h - 1) // TOUT                    # 8 full-ish tiles, then tail
    for t in range(nfull):
        o0 = t * TOUT
        if t == 0:
            tiles.append((0, TOUT, 0, TOUT + PAD, 0))
        else:
            tiles.append((o0, TOUT, o0 - PAD, 2 * PAD + TOUT + (PT - TOUT - 2 * PAD), PAD))
    # tail
    o0 = nfull * TOUT
    orows = h - o0
    i0 = o0 - PAD
    tiles.append((o0, orows, i0, h - i0, PAD))
    return tiles


def _band_blocks_v7(h: int) -> np.ndarray:
    """Stationary band matrices for v7, packed [3, 128, 128] bf16.
    B[slot][r, j]: contribution of in-window row r to PSUM partition j.
    PSUM partition j holds out plane row o0 + (j - xoff), so the PSUM
    output stays partition-aligned with the x tile (engine APs must start
    at partition 0); columns outside the out range are zero."""
    out = np.zeros((3, PT, PT), np.float32)
    tiles = _v7_tiles(h)
    # full conv matrix m[r_in_plane, j_out_plane]
    m = np.zeros((h, h), np.float32)
    for j in range(h):
        for d in range(-PAD, PAD + 1):
            r = j + d
            if r < 0:
                r = -r
            elif r >= h:
                r = 2 * (h - 1) - r
            m[r, j] += 1.0
    # slot 0 from tile 0, slot 1 from tile 1, slot 2 from the tail tile
    for slot, t in ((0, 0), (1, 1), (2, len(tiles) - 1)):
        o0, orows, i0, irows, xoff = tiles[t]
        out[slot, 0:irows, xoff:xoff + orows] = m[i0:i0 + irows,
                                                  o0:o0 + orows]
    return out.astype(ml_dtypes.bfloat16)


def _emit_v7(nc, x_d, bands_d, out_d, planes, h, w, reps=1, hw_loop=0):
    assert w % CHUNK == 0
    nchunks = w // CHUNK
    tiles = _v7_tiles(h)
    nt = len(tiles)

    AF = mybir.ActivationFunctionType
    OP = mybir.AluOpType

    from contextlib import nullcontext

    with tile.TileContext(nc) as tc:
        with (
            tc.tile_pool(name="consts", bufs=1) as cp,
            tc.tile_pool(name="xin", bufs=6) as xp,
            tc.tile_pool(name="thr", bufs=5) as thp,
            tc.tile_pool(name="wsum", bufs=10) as wp,
            tc.tile_pool(name="absp", bufs=5) as ap_pool,
            tc.tile_pool(name="outp", bufs=5) as op_pool,
            tc.tile_pool(name="psum", bufs=4, space="PSUM") as psp,
        ):
            bands_sb = cp.tile([PT, 3, PT], BF16)
            nc.sync.dma_start(bands_sb[:], bands_d.rearrange("m i j -> i m j"))
            bias_thr = cp.tile([PT, 1], F32)
            nc.gpsimd.memset(bias_thr[:], SIGN_BIAS)
            bias_25 = cp.tile([PT, 1], F32)
            nc.gpsimd.memset(bias_25[:], 25.0)

            loop_cm = tc.For_i(0, hw_loop) if hw_loop > 0 else nullcontext()
            with loop_cm:
              for p in [pp for _ in range(reps) for pp in range(planes)]:
                for t, (o0, orows, i0, irows, xoff) in enumerate(tiles):
                    slot = 0 if t == 0 else (2 if t == nt - 1 else 1)
                    B = bands_sb[0:irows, slot, 0:irows]

                    xt = xp.tile([PT, w], F32, tag="x")
                    nc.sync.dma_start(xt[0:irows, :], x_d[p, i0:i0 + irows, :])

                    ce = thp.tile([PT, w + 6], BF16, tag="ce")
                    nc.scalar.activation(ce[0:irows, 3:w + 3], xt[0:irows, :],
                                         AF.Sign, bias=bias_thr[0:irows])
                    # reflect pad in W (3 cols each side) - tiny DVE copies
                    nc.vector.tensor_copy(ce[0:irows, 0:3],
                                          ce[0:irows, 6:3:-1])
                    nc.vector.tensor_copy(ce[0:irows, w + 3:w + 6],
                                          ce[0:irows, w + 1:w - 2:-1])

                    # s1 split: optionally offload left block to GPSIMD
                    # (S1_SPLIT=0: all on DVE; HW gpsimd TT adds measured
                    # ~3.7us/op - far above the cost model - so keep 0)
                    s1 = wp.tile([PT, w + 4], BF16, tag="s1")
                    sp = S1_SPLIT
                    if sp > 0:
                        nc.gpsimd.tensor_tensor(s1[0:irows, 0:sp],
                                                ce[0:irows, 0:sp],
                                                ce[0:irows, 1:sp + 1], OP.add)
                    nc.vector.tensor_tensor(s1[0:irows, sp:w + 4],
                                            ce[0:irows, sp:w + 4],
                                            ce[0:irows, sp + 1:w + 5], OP.add)
                    s2 = wp.tile([PT, w], BF16, tag="s2")
                    nc.vector.tensor_tensor(s2[0:irows, :],
                                            s1[0:irows, 0:w],
                                            s1[0:irows, 2:w + 2], OP.add)

                    ps = psp.tile([PT, nchunks, CHUNK], F32, tag="ps")
                    chains = [(s2, 0), (s1, 4), (ce, 6)]
                    for c in range(nchunks):
                        for k, (src, off) in enumerate(chains):
                            nc.tensor.matmul(
                                ps[0:irows, c, :], B,
                                src[0:irows,
                                    c * CHUNK + off:c * CHUNK + off + CHUNK],
                                start=(k == 0), stop=(k == len(chains) - 1))

                    a = ap_pool.tile([PT, w], BF16, tag="a")
                    nc.scalar.activation(
                        a[0:irows, :], ps[0:irows, :],
                        AF.Abs, bias=bias_25[0:irows])

                    ot = op_pool.tile([PT, w], BF16, tag="ot")
                    nc.vector.scalar_tensor_tensor(
                        ot[0:irows, :], a[0:irows, :], 15.0,
                        xt[0:irows, :], OP.is_gt, OP.mult)
                    nc.sync.dma_start(out_d[p, o0:o0 + orows, :],
                                      ot[xoff:xoff + orows, :])


# ---------------------------------------------------------------------------
# v8: like v7 but EVERY DMA spans exactly 128 partitions (measured: partial-
# partition DMAs run ~53 GB/s vs ~350-374 GB/s for full-128 transfers).
#   - in-windows: t0 rows 0..127, mids 122t-3..122t+124, t8 rows 896..1023
#   - out: packed dram [planes, 9, 128, w] bf16; each tile writes its full
#     128 partitions (3-row boundary garbage included); the host slices the
#     valid rows during assembly.
# ---------------------------------------------------------------------------


def _v8_tiles(h: int):
    """(i0, valid_lo, valid_hi) per tile; in-window = rows i0..i0+127."""
    nfull = (h - 1) // TOUT                    # 8
    tiles = [(0, 0, 124)]
    for t in range(1, nfull):
        tiles.append((TOUT * t - PAD, 3, 124))
    tiles.append((h - PT, PT - (h - TOUT * nfull), PT - 1))   # (896, 80, 127)
    return tiles


def _band_blocks_v8(h: int) -> np.ndarray:
    """[3, 128, 128] bf16: B[slot][r, j] = multiplicity of window row r in
    the 7-tap reflect window of out row i0+j; cols outside valid range 0."""
    out = np.zeros((3, PT, PT), np.float32)
    m = np.zeros((h, h), np.float32)
    for j in range(h):
        for d in range(-PAD, PAD + 1):
            r = j + d
            if r < 0:
                r = -r
            elif r >= h:
                r = 2 * (h - 1) - r
            m[r, j] += 1.0
    tiles = _v8_tiles(h)
    for slot, t in ((0, 0), (1, 1), (2, len(tiles) - 1)):
        i0, vlo, vhi = tiles[t]
        out[slot, :, vlo:vhi + 1] = m[i0:i0 + PT, i0 + vlo:i0 + vhi + 1]
    return out.astype(ml_dtypes.bfloat16)


def _emit_v8(nc, x_d, bands_d, out_d, planes, h, w, reps=1, hw_loop=0,
             pe_chains=3, ablate=""):
    assert w % CHUNK == 0
    nchunks = w // CHUNK
    tiles = _v8_tiles(h)
    nt = len(tiles)

    AF = mybir.ActivationFunctionType
    OP = mybir.AluOpType

    from contextlib import nullcontext

    with tile.TileContext(nc) as tc:
        with (
            tc.tile_pool(name="consts", bufs=1) as cp,
            tc.tile_pool(name="xin", bufs=12) as xp,
            tc.tile_pool(name="thr", bufs=5) as thp,
            tc.tile_pool(name="wsum", bufs=10) as wp,
            tc.tile_pool(name="absp", bufs=5) as ap_pool,
            tc.tile_pool(name="outp", bufs=5) as op_pool,
            tc.tile_pool(name="psum", bufs=4, space="PSUM") as psp,
        ):
            bands_sb = cp.tile([PT, 3, PT], BF16)
            nc.sync.dma_start(bands_sb[:], bands_d.rearrange("m i j -> i m j"))
            bias_thr = cp.tile([PT, 1], F32)
            nc.gpsimd.memset(bias_thr[:], SIGN_BIAS)
            bias_25 = cp.tile([PT, 1], F32)
            nc.gpsimd.memset(bias_25[:], 25.0)

            loop_cm = tc.For_i(0, hw_loop) if hw_loop > 0 else nullcontext()
            with loop_cm:
              for p in [pp for _ in range(reps) for pp in range(planes)]:
                # hoist the plane's reads: R-burst then W-burst per plane
                # cuts HBM read/write turnarounds ~9x
                xts = []
                for ti, (i0, vlo, vhi) in enumerate(tiles):
                    xt = xp.tile([PT, w], F32, tag="x")
                    eng = nc.scalar if (ablate == "2ring" and ti % 2) \
                        else nc.sync
                    eng.dma_start(xt[:], x_d[p, i0:i0 + PT, :])
                    xts.append(xt)
                for t, (i0, vlo, vhi) in enumerate(tiles):
                    slot = 0 if t == 0 else (2 if t == nt - 1 else 1)
                    B = bands_sb[:, slot, :]
                    xt = xts[t]

                    if ablate == "dma":
                        ot = op_pool.tile([PT, w], BF16, tag="ot")
                        nc.vector.tensor_copy(ot[:, 0:8], xt[:, 0:8])
                        nc.sync.dma_start(out_d[p, t, :, :], ot[:])
                        continue

                    ce = thp.tile([PT, w + 6], BF16, tag="ce")
                    nc.scalar.activation(ce[:, 3:w + 3], xt[:],
                                         AF.Sign, bias=bias_thr[:])
                    nc.vector.tensor_copy(ce[:, 0:3], ce[:, 6:3:-1])
                    nc.vector.tensor_copy(ce[:, w + 3:w + 6],
                                          ce[:, w + 1:w - 2:-1])

                    s1 = wp.tile([PT, w + 4], BF16, tag="s1")
                    nc.vector.tensor_tensor(s1[:], ce[:, 0:w + 4],
                                            ce[:, 1:w + 5], OP.add)
                    # pe_chains: 3 = s2 on DVE; 4 = s2 folded into PE;
                    # 34 = alternate by tile parity (balances DVE vs PE)
                    nch = pe_chains if pe_chains in (3, 4) else (3, 4)[t % 2]
                    if nch == 3:
                        s2 = wp.tile([PT, w], BF16, tag="s2")
                        nc.vector.tensor_tensor(s2[:], s1[:, 0:w],
                                                s1[:, 2:w + 2], OP.add)
                        chains = [(s2, 0), (s1, 4), (ce, 6)]
                    else:
                        chains = [(s1, 0), (s1, 2), (s1, 4), (ce, 6)]

                    ps = psp.tile([PT, nchunks, CHUNK], F32, tag="ps")
                    for c in range(nchunks):
                        for k, (src, off) in enumerate(chains):
                            nc.tensor.matmul(
                                ps[:, c, :], B,
                                src[:, c * CHUNK + off:c * CHUNK + off + CHUNK],
                                start=(k == 0), stop=(k == len(chains) - 1))

                    a = ap_pool.tile([PT, w], BF16, tag="a")
                    nc.scalar.activation(
                        a[:], ps.rearrange("q c k -> q (c k)"),
                        AF.Abs, bias=bias_25[:])

                    ot = op_pool.tile([PT, w], BF16, tag="ot")
                    if ablate == "blend":
                        nc.vector.tensor_copy(ot[:, 0:8], a[:, 0:8])
                    else:
                        nc.vector.scalar_tensor_tensor(
                            ot[:], a[:], 15.0, xt[:], OP.is_gt, OP.mult)
                    nc.sync.dma_start(out_d[p, t, :, :], ot[:])


def _assemble_v8(packed: np.ndarray, h: int = H) -> np.ndarray:
    """packed [planes, 9, 128, w] -> [planes, h, w] (f32)."""
    tiles = _v8_tiles(h)
    planes, _, _, w = packed.shape
    out = np.empty((planes, h, w), np.float32)
    out[:, 0:TOUT] = packed[:, 0, 0:TOUT]
    for t in range(1, len(tiles) - 1):
        out[:, TOUT * t:TOUT * (t + 1)] = packed[:, t, PAD:PAD + TOUT]
    i0, vlo, _ = tiles[-1]
    out[:, i0 + vlo:h] = packed[:, len(tiles) - 1, vlo:PT]
    return out


def build_module(planes: int = PLANES, h: int = H, w: int = W,
                 version: int = 4) -> bass.Bass:
    """Standalone module for run_bass_kernel_spmd."""
    nc = Bacc()
    x_d = nc.dram_tensor("x", [planes, h, w], F32, kind="ExternalInput")
    if version >= 8:
        bands_d = nc.dram_tensor("bands", [3, PT, PT], BF16,
                                 kind="ExternalInput")
        out_d = nc.dram_tensor("out", [planes, NT7, PT, w], BF16,
                               kind="ExternalOutput")
        _emit_v8(nc, x_d, bands_d, out_d, planes, h, w)
    elif version == 7:
        bands_d = nc.dram_tensor("bands", [3, PT, PT], BF16,
                                 kind="ExternalInput")
        out_d = nc.dram_tensor("out", [planes, h, w], BF16,
                               kind="ExternalOutput")
        _emit_v7(nc, x_d, bands_d, out_d, planes, h, w)
    else:
        bands_d = nc.dram_tensor("bands", [5, PT, PT], BF16,
                                 kind="ExternalInput")
        out_d = nc.dram_tensor("out", [planes, h, w], F32,
                               kind="ExternalOutput")
        emit = {1: _emit, 2: _emit_v2, 3: _emit_v6, 4: _emit_v5}[version]
        emit(nc, x_d, bands_d, out_d, planes, h, w)
    nc.finalize()
    return nc


VERSION = 8

_MODULE: bass.Bass | None = None


def _get_module() -> bass.Bass:
    global _MODULE
    if _MODULE is None:
        _MODULE = build_module(version=VERSION)
    return _MODULE


def _bands_for(version: int) -> np.ndarray:
    if version >= 8:
        return np.ascontiguousarray(_band_blocks_v8(H))
    if version == 7:
        return np.ascontiguousarray(_band_blocks_v7(H))
    return np.ascontiguousarray(_band_blocks(H))


def _shard_inputs(x: np.ndarray, version: int = None) -> list[dict[str, np.ndarray]]:
    bands = _bands_for(VERSION if version is None else version)
    in_maps = []
    for i in range(N_CORES):
        shard = np.ascontiguousarray(
            x[i * IMGS_PER_CORE:(i + 1) * IMGS_PER_CORE].reshape(PLANES, H, W))
        in_maps.append({"x": shard, "bands": bands})
    return in_maps


def run_sharded(x: np.ndarray, **spmd_kwargs):
    """Compile+run on cores 0..7; returns (full_output, BassKernelResults)."""
    nc = _get_module()
    res = run_bass_kernel_spmd(nc, _shard_inputs(x),
                               core_ids=list(range(N_CORES)), **spmd_kwargs)
    out = np.empty((B, C, H, W), np.float32)
    for i in range(N_CORES):
        r = np.asarray(res.results[i]["out"]).astype(np.float32)
        if VERSION >= 8:
            r = _assemble_v8(r.reshape(PLANES, NT7, PT, W))
        out[i * IMGS_PER_CORE:(i + 1) * IMGS_PER_CORE] = (
            r.reshape(IMGS_PER_CORE, C, H, W))
    return out, res


def kernel(x) -> np.ndarray:
    x = np.asarray(x, dtype=np.float32)
    assert x.shape == (B, C, H, W), x.shape
    out, _ = run_sharded(x)
    return out


# ---------------------------------------------------------------------------
# Timing harness: chained on-device execution via bass_jit + shard_map.
# (The axon client in this container has no NTFF hook, so HW kernel time is
# measured as per-iteration wall time of a long on-device dependency chain.)
# ---------------------------------------------------------------------------

def measure_kernel_ns(x: np.ndarray, t_hi: int = 129, rounds: int = 5,
                      n_per: int = 2, body_reps=(1, 3)) -> float:
    """On-device kernel time via hardware-loop amplification.

    Two NEFFs with IDENTICAL instruction streams wrap the whole pass in a
    For_i hardware loop with trip counts 1 and t_hi; per-pass time =
    (wall(t_hi) - wall(1)) / (t_hi - 1).  The ~90ms axon dispatch overhead
    and any NEFF-size-proportional runtime overhead are identical for both
    and cancel exactly.  Run with two body-reps values to also cancel the
    per-iteration For_i barrier cost:
      M_R = barrier + R * pass  =>  pass = (M_R2 - M_R1) / (R2 - R1).
    """
    import time
    import jax
    import jax.numpy as jnp
    from jax.sharding import Mesh, PartitionSpec
    from concourse import bass2jax

    devices = jax.devices()[:N_CORES]
    mesh = Mesh(np.asarray(devices), ("core",))
    P = PartitionSpec

    def make(T, R):
        @bass2jax.bass_jit
        def _k(nc, xin, bandsin):
            out_d = nc.dram_tensor("out", [PLANES, NT7, PT, W], BF16,
                                   kind="ExternalOutput")
            _emit_v8(nc, xin, bandsin, out_d, PLANES, H, W, reps=R,
                     hw_loop=T)
            return out_d
        return bass2jax.bass_shard_map(_k, mesh=mesh,
                                       in_specs=(P("core"), P("core")),
                                       out_specs=P("core"))

    xg = jnp.asarray(
        np.concatenate([m["x"] for m in _shard_inputs(x)], axis=0))
    bg = jnp.asarray(np.concatenate([_bands_for(VERSION)] * N_CORES, axis=0))

    R1, R2 = body_reps
    fns = {}
    for R in (R1, R2):
        fns[R] = (make(1, R), make(t_hi, R))
        for f in fns[R]:
            y = f(xg, bg)
            y.block_until_ready()

    def timed(f):
        best = np.inf
        for _ in range(n_per):
            t0 = time.perf_counter()
            y = f(xg, bg)
            y.block_until_ready()
            best = min(best, time.perf_counter() - t0)
        return best

    M = {R: [] for R in (R1, R2)}
    for _ in range(rounds):
        for R in (R1, R2):
            f1, fh = fns[R]
            M[R].append((timed(fh) - timed(f1)) / (t_hi - 1))
    m1 = float(np.median(np.array(M[R1])) * 1e9)
    m2 = float(np.median(np.array(M[R2])) * 1e9)
    return (m2 - m1) / (R2 - R1)


def bench_chain(x: np.ndarray, iters: int = 32, warmup: int = 4,
                reps: int = 1):
    import time
    import jax
    import jax.numpy as jnp
    from jax.sharding import Mesh, PartitionSpec
    from concourse import bass2jax

    @bass2jax.bass_jit
    def _jit_kernel(nc, xin, bandsin):
        out_d = nc.dram_tensor("out", [PLANES, H, W], F32,
                               kind="ExternalOutput")
        _emit(nc, xin, bandsin, out_d, PLANES, H, W, reps=reps)
        return out_d

    devices = jax.devices()[:N_CORES]
    mesh = Mesh(np.asarray(devices), ("core",))
    P = PartitionSpec
    f = bass2jax.bass_shard_map(_jit_kernel, mesh=mesh,
                                in_specs=(P("core"), P("core")),
                                out_specs=P("core"))

    xg = jnp.asarray(
        np.concatenate([m["x"] for m in _shard_inputs(x)], axis=0))
    bg = jnp.asarray(np.concatenate([_band_blocks(H)] * N_CORES, axis=0))

    y = f(xg, bg)
    y.block_until_ready()
    for _ in range(warmup):
        y = f(y, bg)
    y.block_until_ready()

    t0 = time.perf_counter()
    for _ in range(iters):
        y = f(y, bg)
    y.block_until_ready()
    dt = (time.perf_counter() - t0) / iters
    return dt, np.asarray(y)

